# revision 2
# baseline (speedup 1.0000x reference)
"""Trainium2 Bass kernel v2 for nn_EqPropTuned (equilibrium-propagation).

Differences vs v1 (baseline):
  - weight-moving matmuls: stationary = state mirror tile [dim_in, batch],
    moving = weight slice [dim_in, 512 out-dims] -> 4x wider moving operand,
    4x fewer matmul instructions.
  - weights pre-scaled by LR=0.3 (fp16) so the whole update
    u = 0.7*s + 0.3*(fwd + bwd + b) accumulates in one PSUM group; the bias
    row and the c1 constant enter PSUM as K=1 / identity matmuls.
  - masters s[l] kept fp32 in natural [batch, dim] layout; fp16 transposed
    mirrors maintained via PE-transposes + ACT copies (no gpsimd anywhere).
  - all DMA transfers are contiguous [P, 4096] pair slabs.

Sharding: data-parallel over batch across 8 cores (128 rows/core), weights
replicated.
"""

import os
import numpy as np
from contextlib import ExitStack

import concourse.bass as bass
import concourse.tile as tile
from concourse import mybir, bacc
from concourse.bass_utils import run_bass_kernel_spmd

F32 = mybir.dt.float32
F16 = mybir.dt.float16
AL = mybir.AluOpType
AF = mybir.ActivationFunctionType

P = 128
CH = 512                              # psum chunk width (one bank)
DIMS = [2048, 2048, 2048, 2048, 1000]
PD = [2048, 2048, 2048, 2048, 1024]   # padded dims
KT = [d // P for d in PD]             # k-tiles per dim [16,16,16,16,8]
BATCH = 1024
N_CORES = 8
BPC = BATCH // N_CORES
N_RELAX = int(os.environ.get("KERNEL_N_RELAX", "25"))
LR = np.float32(0.3)
INV_LR = float(1.0 / np.float32(0.3))

N_W2B_RES_PAIRS = 5                   # w2b k-pairs resident (of 8)


def _pairs(W, Kp, Mp, scale):
    """[Kp//(2P), P, 2*Mp] fp16 pair slabs of scale*W (padded).

    pair i columns [0:Mp] hold W[(2i)P+p, :], columns [Mp:2Mp] hold
    W[(2i+1)P+p, :].
    """
    Wp = np.zeros((Kp, Mp), np.float32)
    Wp[: W.shape[0], : W.shape[1]] = W * scale
    t = Wp.reshape(Kp // P // 2, 2, P, Mp).transpose(0, 2, 1, 3)
    return np.ascontiguousarray(t).reshape(Kp // P // 2, P, 2 * Mp).astype(np.float16)


def build_nc():
    nc = bacc.Bacc(None, target_bir_lowering=False, debug=False)

    d_x = nc.declare_dram_parameter("xT16", [P, PD[0]], F16, isOutput=False)
    d_cx = nc.declare_dram_parameter("cxT16", [P, PD[0]], F16, isOutput=False)
    d_w0f = nc.declare_dram_parameter("w0f", [8, P, 2 * PD[1]], F16, isOutput=False)
    d_w1f = nc.declare_dram_parameter("w1f", [8, P, 2 * PD[2]], F16, isOutput=False)
    d_w2f = nc.declare_dram_parameter("w2f", [8, P, 2 * PD[3]], F16, isOutput=False)
    d_w1b = nc.declare_dram_parameter("w1b", [8, P, 2 * PD[1]], F16, isOutput=False)
    d_w2b = nc.declare_dram_parameter("w2b", [8, P, 2 * PD[2]], F16, isOutput=False)
    d_w3f = nc.declare_dram_parameter("w3f", [8, P, 2 * PD[4]], F16, isOutput=False)
    d_w3b = nc.declare_dram_parameter("w3b", [4, P, 2 * PD[3]], F16, isOutput=False)
    d_brow = {}
    for l in range(1, 5):
        d_brow[l] = nc.declare_dram_parameter(f"brow{l}", [1, PD[l]], F16, isOutput=False)
    d_id32 = nc.declare_dram_parameter("ident32", [P, P], F32, isOutput=False)
    d_id16 = nc.declare_dram_parameter("ident16", [P, P], F16, isOutput=False)
    d_ones = nc.declare_dram_parameter("ones16", [1, P], F16, isOutput=False)
    d_out = nc.declare_dram_parameter("out", [P, PD[4]], F32, isOutput=True)

    with tile.TileContext(nc) as tc, ExitStack() as ctx:
        st = ctx.enter_context(tc.tile_pool(name="state", bufs=1))
        wp = ctx.enter_context(tc.tile_pool(name="wstream", bufs=4))
        pmm = ctx.enter_context(tc.tile_pool(name="pmm", bufs=4, space="PSUM"))
        ptr = ctx.enter_context(tc.tile_pool(name="ptr", bufs=4, space="PSUM"))
        tp = ctx.enter_context(tc.tile_pool(name="tmp", bufs=3))

        s = {}
        sT = {}
        for l in range(1, 5):
            s[l] = st.tile([P, PD[l]], F32, tag=f"s{l}", name=f"s{l}")
            sT[l] = st.tile([P, PD[l]], F16, tag=f"sT{l}", name=f"sT{l}")
        c1s = st.tile([P, PD[1]], F16, tag="c1s", name="c1s")
        brow = {}
        for l in range(1, 5):
            brow[l] = st.tile([1, PD[l]], F16, tag=f"brow{l}", name=f"brow{l}")
            nc.sync.dma_start(brow[l][:], d_brow[l][:])
        id32 = st.tile([P, P], F32, tag="id32", name="id32")
        id16 = st.tile([P, P], F16, tag="id16", name="id16")
        ones = st.tile([1, P], F16, tag="ones", name="ones")
        nc.sync.dma_start(id32[:], d_id32[:])
        nc.sync.dma_start(id16[:], d_id16[:])
        nc.sync.dma_start(ones[:], d_ones[:])

        # input mirrors cycle through the stream pool (slots recycled after init)
        xT = wp.tile([P, PD[0]], F16, tag="slab", name="xT")
        cxT = wp.tile([P, PD[0]], F16, tag="slab", name="cxT")
        nc.sync.dma_start(xT[:], d_x[:])
        nc.sync.dma_start(cxT[:], d_cx[:])

        w2b_res = st.tile([P, N_W2B_RES_PAIRS * 2 * PD[2]], F16, tag="w2b_res",
                          name="w2b_res")
        w3f_res = st.tile([P, 8 * 2 * PD[4]], F16, tag="w3f_res", name="w3f_res")
        w3b_res = st.tile([P, 4 * 2 * PD[3]], F16, tag="w3b_res", name="w3b_res")
        for i in range(N_W2B_RES_PAIRS):
            nc.sync.dma_start(w2b_res[:, i * 2 * PD[2]:(i + 1) * 2 * PD[2]], d_w2b[i])
        for i in range(8):
            nc.sync.dma_start(w3f_res[:, i * 2 * PD[4]:(i + 1) * 2 * PD[4]], d_w3f[i])
        for i in range(4):
            nc.sync.dma_start(w3b_res[:, i * 2 * PD[3]:(i + 1) * 2 * PD[3]], d_w3b[i])

        def res_slice(res, pair_w, k):
            m = pair_w // 2
            base = (k // 2) * pair_w + (k % 2) * m
            return res[:, base:base + m]

        def epilogue(l, c, pm, init):
            off = c * CH
            u = tp.tile([P, CH], F32, tag="u")
            if init:
                # s = clip(psum / 0.3)
                nc.vector.tensor_scalar(u[:], pm[:], INV_LR, 0.0, AL.mult, AL.max)
                nc.vector.tensor_scalar_min(s[l][:, off:off + CH], u[:], 1.0)
            else:
                # s = clip(0.7 s + psum)
                nc.vector.scalar_tensor_tensor(
                    u[:], s[l][:, off:off + CH], 0.7, pm[:], AL.mult, AL.add)
                nc.vector.tensor_scalar(
                    s[l][:, off:off + CH], u[:], 0.0, 1.0, AL.max, AL.min)
            for j in range(4):
                blk = off + j * P
                pt = ptr.tile([P, P], F32, tag="pt")
                nc.tensor.transpose(pt[:], s[l][:, blk:blk + P], id32[:])
                nc.scalar.activation(sT[l][:, blk:blk + P], pt[:], AF.Copy)

        def layer(l, terms, init=False, c1_pass=False):
            """One layer update.

            terms: list of dicts with keys
              stat : stationary mirror AP ([dim_in sub, batch] tiles)
              kt   : number of contraction tiles
              stream: (dram_pairs, n_pairs) or None
              res  : (resident tile, pair_width) or None
              res_upto: first streamed k (0 if fully streamed/resident)
            """
            nch = PD[l] // CH
            pms = [pmm.tile([P, CH], F32, tag="pm", name=f"pm{c}")
                   for c in range(nch)]
            for c in range(nch):
                if (not init) and l == 1:
                    nc.tensor.matmul(pms[c][:], id16[:], c1s[:, c * CH:(c + 1) * CH],
                                     start=True, stop=False)
                else:
                    nc.tensor.matmul(pms[c][:], ones[:], brow[l][:, c * CH:(c + 1) * CH],
                                     start=True, stop=False)

            def do_k(stat, k, wsl, k_last):
                for c in range(nch):
                    nc.tensor.matmul(
                        pms[c][:], stat[:, bass.ts(k, P)],
                        wsl[:, c * CH:(c + 1) * CH],
                        start=False, stop=k_last)

            for ti, term in enumerate(terms):
                t_last = ti == len(terms) - 1
                stat, kt = term["stat"], term["kt"]
                res_upto = term.get("res_upto", 0)
                M = PD[l]
                if term.get("res") is not None:
                    res, pair_w = term["res"]
                    k_end = res_upto if term.get("stream") else kt
                    for k in range(k_end):
                        do_k(stat, k, res_slice(res, pair_w, k),
                             t_last and k == kt - 1)
                if term.get("stream") is not None:
                    dram, n_pairs = term["stream"]
                    for i in range(res_upto // 2, n_pairs):
                        w = wp.tile([P, 2 * M], F16, tag="slab", name="wpair")
                        nc.sync.dma_start(w[:], dram[i])
                        for k2 in range(2):
                            k = 2 * i + k2
                            do_k(stat, k, w[:, k2 * M:(k2 + 1) * M],
                                 t_last and k == kt - 1)
            if c1_pass:
                for c in range(nch):
                    nc.scalar.activation(c1s[:, c * CH:(c + 1) * CH], pms[c][:],
                                         AF.Copy)
            else:
                for c in range(nch):
                    epilogue(l, c, pms[c], init)

        # ---- init ----
        # s1 = clip(x @ W0 + b1): psum = x @ W0s + 0.3 b1, then /0.3 + clip
        layer(1, [{"stat": xT, "kt": KT[0], "stream": (d_w0f, 8)}], init=True)
        # c1s = clip(x) @ W0s + 0.3 b1 (W0 streamed a second time)
        layer(1, [{"stat": cxT, "kt": KT[0], "stream": (d_w0f, 8)}],
              init=True, c1_pass=True)
        layer(2, [{"stat": sT[1], "kt": KT[1], "stream": (d_w1f, 8)}], init=True)
        layer(3, [{"stat": sT[2], "kt": KT[2], "stream": (d_w2f, 8)}], init=True)
        layer(4, [{"stat": sT[3], "kt": KT[3], "res": (w3f_res, 2 * PD[4])}],
              init=True)

        # ---- relaxation sweeps ----
        for _ in range(N_RELAX):
            layer(1, [{"stat": sT[2], "kt": KT[1], "stream": (d_w1b, 8)}])
            layer(2, [
                {"stat": sT[1], "kt": KT[1], "stream": (d_w1f, 8)},
                {"stat": sT[3], "kt": KT[3], "res": (w2b_res, 2 * PD[2]),
                 "res_upto": 2 * N_W2B_RES_PAIRS, "stream": (d_w2b, 8)},
            ])
            layer(3, [
                {"stat": sT[2], "kt": KT[2], "stream": (d_w2f, 8)},
                {"stat": sT[4], "kt": KT[4], "res": (w3b_res, 2 * PD[3])},
            ])
            layer(4, [{"stat": sT[3], "kt": KT[3], "res": (w3f_res, 2 * PD[4])}])

        nc.sync.dma_start(d_out[:], s[4][:])

    nc.compile()
    return nc


def _prep_inputs(x, W0, W1, W2, W3, b1, b2, b3, b4):
    common = {
        "w0f": _pairs(W0, PD[0], PD[1], LR),
        "w1f": _pairs(W1, PD[1], PD[2], LR),
        "w2f": _pairs(W2, PD[2], PD[3], LR),
        "w3f": _pairs(W3, PD[3], PD[4], LR),
        "w1b": _pairs(np.ascontiguousarray(W1.T), PD[2], PD[1], LR),
        "w2b": _pairs(np.ascontiguousarray(W2.T), PD[3], PD[2], LR),
        "w3b": _pairs(np.ascontiguousarray(W3.T), PD[4], PD[3], LR),
        "ident32": np.eye(P, dtype=np.float32),
        "ident16": np.eye(P, dtype=np.float16),
        "ones16": np.ones((1, P), np.float16),
    }
    for l, b in zip(range(1, 5), [b1, b2, b3, b4]):
        br = np.zeros((1, PD[l]), np.float32)
        br[0, : b.shape[0]] = b * LR
        common[f"brow{l}"] = br.astype(np.float16)

    in_maps = []
    for c in range(N_CORES):
        xs = np.asarray(x[c * BPC:(c + 1) * BPC], dtype=np.float32)
        # xT[p, k*P + j] = xs[j, k*P + p]
        xT = np.ascontiguousarray(
            xs.reshape(BPC, PD[0] // P, P).transpose(2, 1, 0)
        ).reshape(P, PD[0])
        in_maps.append({
            "xT16": xT.astype(np.float16),
            "cxT16": np.clip(xT, 0.0, 1.0).astype(np.float16),
            **common,
        })
    return in_maps


_NC_CACHE = None


def _get_nc():
    global _NC_CACHE
    if _NC_CACHE is None:
        _NC_CACHE = build_nc()
    return _NC_CACHE


def run(inputs, trace=False):
    nc = _get_nc()
    in_maps = _prep_inputs(**inputs)
    res = run_bass_kernel_spmd(nc, in_maps, list(range(N_CORES)), trace=trace)
    outs = []
    for c in range(N_CORES):
        o = res.results[c]["out"]      # [128 batch rows, 1024 dims]
        outs.append(o[:, : DIMS[4]])
    return np.concatenate(outs, axis=0).astype(np.float32), res


def kernel(**inputs):
    out, _ = run(inputs, trace=False)
    return out
